# revision 1
# baseline (speedup 1.0000x reference)
"""Guide-token attention kernel for Trainium2 (8 NeuronCores).

Module: y[b] = softmax(((Q+tQ) @ (K+tK)^T)/sqrt(hd)) @ V  per head, where
  Q = x @ Wq^T + bq, K = x @ Wk^T + bk, V = x @ Wv^T + bv,
  tQ/tK are projections of a per-batch guide token (broadcast over seq).

Shapes: x [4, 1024, 1024], tokens [4, 1, 1024], W* [1024, 1024], b* [1024].
H=16 heads, hd=64.

Sharding: 8 cores = 4 batches x 2 head-groups (8 heads each); weights
column-sharded per head group; each core sees one batch -> no cross-core
communication.

Layout (PE contracts over the partition axis; no on-chip transposes):
  - host pre-transposes x[b] -> xT [D, S] and W slices -> wT [D, 512] (bf16),
    and precomputes the tiny guide-token adds (tq + 2*bq etc.).
  - QT/KT computed transposed [feat, S]; V computed natural [S, feat].
  - scores computed directly transposed per head: sT[k, q] = cK @ cQ^T
    (lhsT = cKT slice, rhs = cQT slice, contraction = hd = 64).
  - exp on ScalarE over two-bank PSUM tiles [128, 2, 512] (amortizes the
    ~352-cycle ACTIVATE overhead), writing bf16 probs. Softmax max-
    subtraction skipped: |scores| <= ~15 so exp is safely in fp32/bf16 range.
  - AV: lhsT = V chunk [k, 64] + appended ones column (row 64 of the output
    accumulates the softmax denominator), rhs = probsT [k, q], accumulated
    over k chunks -> [65, q] PSUM.
  - normalize: denominator row -> SBUF, reciprocal (fast-approx), GpSimd
    partition_broadcast to 64 rows, one VectorE multiply -> yT [feat, S].
  - host reassembles y[b][:, cols] = yT^T and adds bv once at the end
    (softmax rows sum to 1, so y = softmax@V0 + bv exactly).

Schedule (HAM-aware): the PE instruction stream is kept dense so the clock
gate stays at 8/8. QK(ft0) runs first; the h0-h3 attention units then
interleave V and QK(ft1) matmuls as fillers between score pairs (3 filler
MMs per pair exactly consumes the 96 remaining projection MMs at the rate
ACT drains exps); the h4-h7 score units interleave with the h0-h3 AV units.
"""

import os

import numpy as np
import ml_dtypes

import concourse.bass as bass
import concourse.tile as tile
from concourse import bacc
from concourse import mybir
from concourse.bass_utils import run_bass_kernel_spmd

B = 4
S = 1024
D = 1024
H = 16
HD = 64
NCORES = 8
FPG = 512          # features per head-group (8 heads * 64)
NKC = D // 128     # contraction chunks for projections
NFT = FPG // 128   # feature tiles per group
NST = S // 128     # sequence tiles
NQB = S // 512     # 512-wide query blocks
HPG = 8            # heads per group
NPAIR = NST // 2   # kt pairs per unit

BF16 = mybir.dt.bfloat16
F32 = mybir.dt.float32

_CACHE = {}


def _build():
    nc = bacc.Bacc()

    # Inputs pre-shuffled on host to [128 partitions, kc, cols] so HBM order
    # matches SBUF order: per-partition-contiguous 16KB/8KB DMA packets
    # instead of 2KB (DMA is packet-bound otherwise).
    xT = nc.declare_dram_parameter("xT", [128, NKC, S], BF16, isOutput=False)
    wqT = nc.declare_dram_parameter("wqT", [128, NKC, FPG], BF16, isOutput=False)
    wkT = nc.declare_dram_parameter("wkT", [128, NKC, FPG], BF16, isOutput=False)
    wvT = nc.declare_dram_parameter("wvT", [128, NKC, FPG], BF16, isOutput=False)
    qadd = nc.declare_dram_parameter("qadd", [128, NFT], F32, isOutput=False)
    kadd = nc.declare_dram_parameter("kadd", [128, NFT], F32, isOutput=False)
    yT = nc.declare_dram_parameter("yT", [FPG, S], F32, isOutput=True)

    with tile.TileContext(nc) as tc:
        with (
            tc.tile_pool(name="persist", bufs=1) as persist,
            tc.tile_pool(name="probs", bufs=44) as probs_pool,
            tc.tile_pool(name="norm", bufs=4) as norm_pool,
            tc.tile_pool(name="psP", bufs=2, space=bass.MemorySpace.PSUM) as psP,
            tc.tile_pool(name="psA", bufs=2, space=bass.MemorySpace.PSUM) as psA,
            tc.tile_pool(name="psAV", bufs=2, space=bass.MemorySpace.PSUM) as psAV,
        ):
            # ---- persistent SBUF tensors ----
            xt = persist.tile([128, NKC, S], BF16)
            wq = persist.tile([128, NKC, FPG], BF16)
            wk = persist.tile([128, NKC, FPG], BF16)
            wv = persist.tile([128, NKC, FPG], BF16)
            qa = persist.tile([128, NFT], F32)
            ka = persist.tile([128, NFT], F32)
            cq = persist.tile([128, NFT, S], BF16)          # cQT/8  [feat, S]
            ck = persist.tile([128, NFT, S], BF16)          # cKT    [feat, S]
            vt = persist.tile([128, NST, HPG, HD + 1], BF16)  # V' + ones col
            yt = persist.tile([128, NFT, S], F32)           # yT [feat, S]

            # ---- input DMAs (wq/x first: QK ft0 starts the kernel) ----
            # Partition-sliced so transfers parallelize across DMA queues
            # while keeping 16KB per-partition-contiguous packets.
            nc.sync.dma_start(out=qa[:], in_=qadd[:])
            nc.sync.dma_start(out=ka[:], in_=kadd[:])
            for dst, srcp in ((wq, wqT), (xt, xT), (wk, wkT), (wv, wvT)):
                for p in range(4):
                    ps = slice(p * 32, (p + 1) * 32)
                    nc.sync.dma_start(out=dst[ps, :, :], in_=srcp[ps, :, :])

            nc.vector.memset(vt[:, :, :, HD:HD + 1], 1.0)

            # ---- HAM pre-warm: dummy matmuls while input DMAs stream ----
            # The PE clock gate needs ~3.4us of sustained activity to go
            # 8/8; burn the DMA head (~10us) on throwaway matmuls so the
            # real projections start at full clock.
            wrm = persist.tile([128, 512], BF16)
            nc.gpsimd.memset(wrm[:], 0.0)
            wacc = psAV.tile([128, 512], F32, tag="psAV")
            for _ in range(12):
                nc.tensor.matmul(
                    wacc[:], wrm[:, 0:128], wrm[:], start=True, stop=True
                )

            # ---- projection building blocks ----
            def qk_group(which, ft, sb):
                """QT/KT [feat tile, S block] accumulated over D chunks,
                evicted to bf16 with the guide-token add (+1/8 scale for Q)."""
                w_sb, add_sb, scale, dst = (
                    (wq, qa, 0.125, cq) if which == "q" else (wk, ka, 1.0, ck)
                )
                acc = psP.tile([128, 512], F32, tag="psP")
                for kc in range(NKC):
                    yield lambda kc=kc, acc=acc: nc.tensor.matmul(
                        acc[:],
                        w_sb[:, kc, ft * 128:(ft + 1) * 128],
                        xt[:, kc, sb * 512:(sb + 1) * 512],
                        start=(kc == 0),
                        stop=(kc == NKC - 1),
                    )
                yield lambda acc=acc: nc.vector.tensor_scalar(
                    out=dst[:, ft, sb * 512:(sb + 1) * 512],
                    in0=acc[:],
                    scalar1=scale,
                    scalar2=add_sb[:, ft:ft + 1],
                    op0=mybir.AluOpType.mult,
                    op1=mybir.AluOpType.add,
                )

            def v_group(st):
                """V [S tile, feat] natural layout, strided into vt."""
                acc = psP.tile([128, 512], F32, tag="psP")
                for kc in range(NKC):
                    yield lambda kc=kc, acc=acc: nc.tensor.matmul(
                        acc[:],
                        xt[:, kc, st * 128:(st + 1) * 128],
                        wv[:, kc, :],
                        start=(kc == 0),
                        stop=(kc == NKC - 1),
                    )
                yield lambda acc=acc: nc.vector.tensor_copy(
                    out=vt[:, st, :, 0:HD], in_=acc[:]
                )

            def run(gen):
                for op in gen:
                    op()

            # filler stream: QK ft2/ft3 + all of V (96 MMs + evictions)
            def filler_stream():
                for which in ("q", "k"):
                    for ft in (2, 3):
                        for sb in range(NQB):
                            yield from qk_group(which, ft, sb)
                for st in range(NST):
                    yield from v_group(st)

            # ---- attention building blocks ----
            def unit_scores(hp, qb, filler=None):
                """Score MMs for head pair (2hp, 2hp+1), one 512-wide query
                block. The two heads' operands live on partitions 0-63 /
                64-127 -> different PE row groups, so their K=64 matmuls run
                concurrently (row tiling). exp pairs on ScalarE -> bf16
                probs. Pulls filler ops to keep the PE stream dense."""
                ft = hp
                qsl = slice(qb * 512, (qb + 1) * 512)
                pairsA, pairsB = [], []
                for p in range(NPAIR):
                    scA = psA.tile([128, 2, 512], F32, tag="psA")
                    scB = psA.tile([128, 2, 512], F32, tag="psA")
                    for j in range(2):
                        kt = 2 * p + j
                        ksl = slice(kt * 128, (kt + 1) * 128)
                        nc.tensor.matmul(
                            scA[:, j, :], ck[0:64, ft, ksl], cq[0:64, ft, qsl],
                            start=True, stop=True,
                        )
                        nc.tensor.matmul(
                            scB[:, j, :], ck[64:128, ft, ksl], cq[64:128, ft, qsl],
                            start=True, stop=True,
                        )
                    prA = probs_pool.tile([128, 2, 512], BF16, tag="probs")
                    nc.scalar.activation(
                        out=prA[:], in_=scA[:],
                        func=mybir.ActivationFunctionType.Exp,
                    )
                    prB = probs_pool.tile([128, 2, 512], BF16, tag="probs")
                    nc.scalar.activation(
                        out=prB[:], in_=scB[:],
                        func=mybir.ActivationFunctionType.Exp,
                    )
                    pairsA.append(prA)
                    pairsB.append(prB)
                    if filler is not None:
                        for _ in range(8):
                            op = next(filler, None)
                            if op is not None:
                                op()
                return pairsA, pairsB

            def head_av(h, qb, pairs):
                """AV accumulation + softmax normalization -> yt slice."""
                pbase = (h % 2) * 64
                ft = h // 2
                qsl = slice(qb * 512, (qb + 1) * 512)
                av = psAV.tile([HD + 1, 512], F32, tag="psAV")
                for kt in range(NST):
                    nc.tensor.matmul(
                        av[:],
                        vt[:, kt, h, :],
                        pairs[kt // 2][:, kt % 2, :],
                        start=(kt == 0),
                        stop=(kt == NST - 1),
                    )
                den = norm_pool.tile([1, 512], F32, tag="den")
                nc.vector.tensor_copy(out=den[:], in_=av[HD:HD + 1, :])
                rec = norm_pool.tile([1, 512], F32, tag="rec")
                nc.vector.reciprocal_approx_fast(out=rec[:], in_=den[:])
                recb = norm_pool.tile([HD, 512], F32, tag="recb")
                nc.gpsimd.partition_broadcast(recb[:], rec[:])
                nc.vector.tensor_tensor(
                    out=yt[pbase:pbase + 64, ft, qsl],
                    in0=av[0:HD, :],
                    in1=recb[:],
                    op=mybir.AluOpType.mult,
                )

            def unit_av(hp, qb, pr):
                head_av(2 * hp, qb, pr[0])
                head_av(2 * hp + 1, qb, pr[1])

            # ---- schedule ----
            # Phase 1: QK ft0/ft1 dense (head pairs 0-1 depend only on these).
            for which in ("q", "k"):
                for ft in (0, 1):
                    for sb in range(NQB):
                        run(qk_group(which, ft, sb))

            units = [(hp, qb) for hp in range(HPG // 2) for qb in range(NQB)]
            early, late = units[:4], units[4:]

            # Phase 2: early score units with projection fillers.
            filler = filler_stream()
            pairs_of = {}
            for hp, qb in early:
                pairs_of[(hp, qb)] = unit_scores(hp, qb, filler=filler)
            for op in filler:   # drain any remainder (V must precede AV)
                op()

            # Output DMA per feature tile as soon as both its units are done.
            done_units = set()

            def maybe_flush(hp, qb):
                done_units.add((hp, qb))
                if all((hp, q) in done_units for q in range(NQB)):
                    nc.sync.dma_start(
                        out=yT[hp * 128:(hp + 1) * 128, :], in_=yt[:, hp, :]
                    )

            # Phase 3: early AV interleaved with late score units.
            for i, (hp, qb) in enumerate(late):
                unit_av(*early[i], pairs_of.pop(early[i]))
                maybe_flush(*early[i])
                pairs_of[(hp, qb)] = unit_scores(hp, qb)

            # Phase 4: late AV units.
            for hp, qb in late:
                unit_av(hp, qb, pairs_of.pop((hp, qb)))
                maybe_flush(hp, qb)

    nc.finalize()
    return nc


def _get_nc():
    if "nc" not in _CACHE:
        _CACHE["nc"] = _build()
    return _CACHE["nc"]


def kernel(x, tokens, Wq, bq, Wk, bk, Wv, bv):
    x = np.asarray(x, dtype=np.float32)
    tokens = np.asarray(tokens, dtype=np.float32)
    Wq = np.asarray(Wq, dtype=np.float32)
    Wk = np.asarray(Wk, dtype=np.float32)
    Wv = np.asarray(Wv, dtype=np.float32)
    bq = np.asarray(bq, dtype=np.float32)
    bk = np.asarray(bk, dtype=np.float32)
    bv = np.asarray(bv, dtype=np.float32)

    bf16 = ml_dtypes.bfloat16
    in_maps = []
    for c in range(NCORES):
        b, g = divmod(c, 2)
        rows = slice(g * FPG, (g + 1) * FPG)
        tq = tokens[b, 0] @ Wq[rows].T + 2.0 * bq[rows]   # [512]
        tk = tokens[b, 0] @ Wk[rows].T + 2.0 * bk[rows]
        def pack(aT):
            # [D, C] -> [128, NKC, C]: partition-major to match SBUF layout
            return np.ascontiguousarray(
                aT.reshape(NKC, 128, aT.shape[1]).transpose(1, 0, 2)
            ).astype(bf16)

        in_maps.append({
            "xT": pack(x[b].T),
            "wqT": pack(Wq[rows].T),
            "wkT": pack(Wk[rows].T),
            "wvT": pack(Wv[rows].T),
            "qadd": np.ascontiguousarray((tq / 8.0).reshape(NFT, 128).T).astype(np.float32),
            "kadd": np.ascontiguousarray(tk.reshape(NFT, 128).T).astype(np.float32),
        })

    nc = _get_nc()
    trace = bool(int(os.environ.get("KERNEL_TRACE", "0")))
    res = run_bass_kernel_spmd(nc, in_maps, core_ids=list(range(NCORES)), trace=trace)
    if trace:
        _CACHE["last_results"] = res

    y = np.empty((B, S, D), dtype=np.float32)
    for c in range(NCORES):
        b, g = divmod(c, 2)
        y[b, :, g * FPG:(g + 1) * FPG] = res.results[c]["yT"].T
    y += bv[None, None, :]
    return y



# revision 4
# speedup vs baseline: 1.1222x; 1.1222x over previous
"""Guide-token attention kernel for Trainium2 (8 NeuronCores).

Module: y[b] = softmax(((Q+tQ) @ (K+tK)^T)/sqrt(hd)) @ V  per head, where
  Q = x @ Wq^T + bq, K = x @ Wk^T + bk, V = x @ Wv^T + bv,
  tQ/tK are projections of a per-batch guide token (broadcast over seq).

Shapes: x [4, 1024, 1024], tokens [4, 1, 1024], W* [1024, 1024], b* [1024].
H=16 heads, hd=64.

Sharding: 8 cores = 4 batches x 2 head-groups (8 heads each); weights
column-sharded per head group; each core sees one batch -> no cross-core
communication.

Layout (PE contracts over the partition axis; no on-chip transposes):
  - host pre-transposes x[b] -> xT [D, S] and W slices (bf16), and
    precomputes the tiny guide-token adds (tq + 2*bq etc.).
  - QT/KT computed transposed [feat, S]; V computed natural [S, feat].
  - scores computed directly transposed per head: sT[k, q] = cK @ cQ^T
    (lhsT = cKT slice, rhs = cQT slice, contraction = hd = 64); the two
    heads of a pair live on PE row halves -> concurrent streams.
  - exp on ScalarE over two-bank PSUM tiles [128, 2, 512] -> bf16 probs.
    Softmax max-subtraction skipped: |scores| <= ~15, safe in fp32/bf16.
  - AV: lhsT = V chunk [k, 64] + ones column (row 64 accumulates the
    softmax denominator), rhs = probsT [k, q] -> [65, q] PSUM.
  - normalize: denominator row -> SBUF, reciprocal (fast-approx), GpSimd
    partition_broadcast, one VectorE multiply -> bf16 yt; per-(ft,qb)
    output flush.

Schedule (engine-balance aware). ScalarE exp is ~73us total and the PE's
real work is ~82us, so both must run dense from early on:
  - input DMAs are consolidated (adds, w-ft0 pair, xT quarters, the other
    w-fts, wv) so the first projections start a few us in; dummy matmuls
    bridge the HAM clock-gate ramp until data lands, and a dummy exp
    preloads the ACT spline table.
  - "wave A" computes Q/K ft0 kc-outer (4 PSUM accumulators round-robin),
    paced by the arriving xT quarters -> first score unit early.
  - the 8 score units run back-to-back; between exp pairs the PE pulls
    filler work from a deque fed by a per-unit plan: ft1, V, ft2, then
    AV blocks of done units interleaved ahead of ft3 so nothing misses
    its deadline and the tail stays short.
"""

import os
from collections import deque

import numpy as np
import ml_dtypes

import concourse.bass as bass
import concourse.tile as tile
from concourse import bacc
from concourse import mybir
from concourse.bass_utils import run_bass_kernel_spmd

B = 4
S = 1024
D = 1024
H = 16
HD = 64
NCORES = 8
FPG = 512          # features per head-group (8 heads * 64)
NKC = D // 128     # contraction chunks for projections
NFT = FPG // 128   # feature tiles per group
NST = S // 128     # sequence tiles
NQB = S // 512     # 512-wide query blocks
HPG = 8            # heads per group
NPAIR = NST // 2   # kt pairs per unit

BF16 = mybir.dt.bfloat16
F32 = mybir.dt.float32

_CACHE = {}


def _build():
    nc = bacc.Bacc()

    # Inputs pre-shuffled on host so HBM order matches SBUF order, and
    # consolidated so the priority path is few large DMAs.
    xT = nc.declare_dram_parameter("xT", [128, NKC, S], BF16, isOutput=False)
    adds = nc.declare_dram_parameter("adds", [128, 2, NFT], F32, isOutput=False)
    w0 = nc.declare_dram_parameter("w0", [128, 2, NKC, 128], BF16, isOutput=False)
    wqk = nc.declare_dram_parameter("wqk", [128, 2, 3, NKC, 128], BF16, isOutput=False)
    wvT = nc.declare_dram_parameter("wvT", [128, NKC, FPG], BF16, isOutput=False)
    # y blocks [ft, qb] of [128 feat, 512 q], bf16 (host re-expands to f32)
    yT = nc.declare_dram_parameter("yT", [NFT * NQB * 128, 512], BF16, isOutput=True)

    with tile.TileContext(nc) as tc:
        with (
            tc.tile_pool(name="persist", bufs=1) as persist,
            tc.tile_pool(name="probs", bufs=48) as probs_pool,
            tc.tile_pool(name="norm", bufs=2) as norm_pool,
            tc.tile_pool(name="psP", bufs=2, space=bass.MemorySpace.PSUM) as psP,
            tc.tile_pool(name="psA", bufs=2, space=bass.MemorySpace.PSUM) as psA,
            tc.tile_pool(name="psAV", bufs=2, space=bass.MemorySpace.PSUM) as psAV,
        ):
            # ---- persistent SBUF tensors ----
            xt = persist.tile([128, NKC, S], BF16)
            w0sb = persist.tile([128, 2, NKC, 128], BF16)      # wq/wk ft0
            wqksb = persist.tile([128, 2, 3, NKC, 128], BF16)  # wq/wk ft1-3
            wv = persist.tile([128, NKC, FPG], BF16)
            addsb = persist.tile([128, 2, NFT], F32)
            cq = persist.tile([128, NFT, S], BF16)          # cQT/8  [feat, S]
            ck = persist.tile([128, NFT, S], BF16)          # cKT    [feat, S]
            vt = persist.tile([128, NST, HPG, HD + 1], BF16)  # V' + ones col
            yt = persist.tile([128, NFT, S], BF16)          # yT [feat, S]
            wrm = persist.tile([128, 512], BF16)
            dum = persist.tile([1, 8], F32)

            def wsel(which, ft):
                wi = 0 if which == "q" else 1
                if ft == 0:
                    return w0sb[:, wi]
                return wqksb[:, wi, ft - 1]

            # ---- input DMAs, consolidated, priority order ----
            nc.sync.dma_start(out=addsb[:], in_=adds[:])
            nc.sync.dma_start(out=w0sb[:], in_=w0[:])
            for i in range(4):
                nc.sync.dma_start(out=xt[:, 2 * i:2 * i + 2, :],
                                  in_=xT[:, 2 * i:2 * i + 2, :])
            nc.sync.dma_start(out=wqksb[:], in_=wqk[:])
            nc.sync.dma_start(out=wv[:], in_=wvT[:])

            nc.vector.memset(wrm[:], 0.0)
            nc.vector.memset(vt[:, :, :, HD:HD + 1], 1.0)
            # preload the exp spline table while DMAs stream
            nc.scalar.activation(out=dum[:], in_=wrm[0:1, 0:8],
                                 func=mybir.ActivationFunctionType.Exp)

            # ---- HAM pre-warm: dummy matmuls until the first inputs land ----
            wacc = psAV.tile([128, 512], F32, tag="psAV")
            for _ in range(14):
                nc.tensor.matmul(
                    wacc[:], wrm[:, 0:128], wrm[:], start=True, stop=True
                )

            # ---- wave A: Q/K ft0, kc-outer, paced by the xT quarter DMAs ----
            accQ = psA.tile([128, 2, 512], F32, tag="psA")
            accK = psA.tile([128, 2, 512], F32, tag="psA")
            for kc in range(NKC):
                for acc, wi in ((accQ, 0), (accK, 1)):
                    for sb in range(NQB):
                        nc.tensor.matmul(
                            acc[:, sb, :],
                            w0sb[:, wi, kc, :],
                            xt[:, kc, sb * 512:(sb + 1) * 512],
                            start=(kc == 0),
                            stop=(kc == NKC - 1),
                        )
            for sb in range(NQB):
                nc.vector.tensor_scalar(
                    out=cq[:, 0, sb * 512:(sb + 1) * 512], in0=accQ[:, sb, :],
                    scalar1=0.125, scalar2=addsb[:, 0, 0:1],
                    op0=mybir.AluOpType.mult, op1=mybir.AluOpType.add,
                )
            for sb in range(NQB):
                nc.vector.tensor_scalar(
                    out=ck[:, 0, sb * 512:(sb + 1) * 512], in0=accK[:, sb, :],
                    scalar1=1.0, scalar2=addsb[:, 1, 0:1],
                    op0=mybir.AluOpType.mult, op1=mybir.AluOpType.add,
                )

            # ---- filler deque: (pe_credit, op) ----
            fill = deque()

            def qk_group(which, ft, sb):
                wi = 0 if which == "q" else 1
                scale = 0.125 if which == "q" else 1.0
                dst = cq if which == "q" else ck
                w_ap = wsel(which, ft)
                acc = psP.tile([128, 512], F32, tag="psP", name="acc")
                for kc in range(NKC):
                    yield 1, (lambda kc=kc, acc=acc, w_ap=w_ap: nc.tensor.matmul(
                        acc[:],
                        w_ap[:, kc],
                        xt[:, kc, sb * 512:(sb + 1) * 512],
                        start=(kc == 0),
                        stop=(kc == NKC - 1),
                    ))
                yield 0, (lambda acc=acc: nc.vector.tensor_scalar(
                    out=dst[:, ft, sb * 512:(sb + 1) * 512],
                    in0=acc[:],
                    scalar1=scale,
                    scalar2=addsb[:, wi, ft:ft + 1],
                    op0=mybir.AluOpType.mult,
                    op1=mybir.AluOpType.add,
                ))

            def v_group(st):
                acc = psP.tile([128, 512], F32, tag="psP", name="acc")
                for kc in range(NKC):
                    yield 1, (lambda kc=kc, acc=acc: nc.tensor.matmul(
                        acc[:],
                        xt[:, kc, st * 128:(st + 1) * 128],
                        wv[:, kc, :],
                        start=(kc == 0),
                        stop=(kc == NKC - 1),
                    ))
                yield 0, (lambda acc=acc: nc.vector.tensor_copy(
                    out=vt[:, st, :, 0:HD], in_=acc[:]
                ))

            def head_av(h, qb, pairs):
                """AV accumulation + softmax normalization ops for one head."""
                ft = h // 2
                pbase = (h % 2) * 64
                qsl = slice(qb * 512, (qb + 1) * 512)
                av = psAV.tile([HD + 1, 512], F32, tag="psAV", name="av")
                for kt in range(NST):
                    yield 1, (lambda kt=kt, av=av: nc.tensor.matmul(
                        av[:],
                        vt[:, kt, h, :],
                        pairs[kt // 2][:, kt % 2, :],
                        start=(kt == 0),
                        stop=(kt == NST - 1),
                    ))
                den = norm_pool.tile([1, 512], F32, tag="den", name="den")
                yield 0, (lambda av=av, den=den: nc.vector.tensor_copy(
                    out=den[:], in_=av[HD:HD + 1, :]
                ))
                rec = norm_pool.tile([1, 512], F32, tag="rec", name="rec")
                yield 0, (lambda den=den, rec=rec: nc.vector.reciprocal_approx_fast(
                    out=rec[:], in_=den[:]
                ))
                recb = norm_pool.tile([HD, 512], F32, tag="recb", name="recb")
                yield 0, (lambda rec=rec, recb=recb: nc.gpsimd.partition_broadcast(
                    recb[:], rec[:]
                ))
                yield 0, (lambda av=av, recb=recb: nc.vector.tensor_tensor(
                    out=yt[pbase:pbase + 64, ft, qsl],
                    in0=av[0:HD, :],
                    in1=recb[:],
                    op=mybir.AluOpType.mult,
                ))

            def av_block(hp, qb, pairsA, pairsB):
                yield from head_av(2 * hp, qb, pairsA)
                yield from head_av(2 * hp + 1, qb, pairsB)
                row = (hp * NQB + qb) * 128
                qsl = slice(qb * 512, (qb + 1) * 512)
                yield 0, (lambda: nc.sync.dma_start(
                    out=yT[row:row + 128, :], in_=yt[:, hp, qsl]
                ))

            def pull(credits):
                got = 0
                while fill and got < credits:
                    cr, fn = fill.popleft()
                    fn()
                    got += cr

            # per-unit filler plan (each entry = ~32 PE credits):
            #   u0: QK ft1    u1: V 0-3     u2: V 4-7     u3: QK ft2
            #   u4: AV(u0,u1) u5: QK ft3    u6: AV(u2,u3) u7: AV(u4,u5)
            # drain: AV(u6), AV(u7)
            def proj_segment(ft):
                for which in ("q", "k"):
                    for sb in range(NQB):
                        yield from qk_group(which, ft, sb)

            def v_segment(lo, hi):
                for st in range(lo, hi):
                    yield from v_group(st)

            units = [(hp, qb) for hp in range(HPG // 2) for qb in range(NQB)]
            pairs_of = {}
            av_sched = {4: [0, 1], 6: [2, 3], 7: [4, 5]}

            def push_av(uidx):
                hp, qb = units[uidx]
                pp = pairs_of.pop(units[uidx])
                fill.extend(av_block(hp, qb, pp[0], pp[1]))

            for ui, (hp, qb) in enumerate(units):
                if ui == 0:
                    fill.extend(proj_segment(1))
                elif ui == 1:
                    fill.extend(v_segment(0, 4))
                elif ui == 2:
                    fill.extend(v_segment(4, 8))
                elif ui == 3:
                    fill.extend(proj_segment(2))
                elif ui == 5:
                    fill.extend(proj_segment(3))
                for u in av_sched.get(ui, ()):
                    push_av(u)

                ft = hp
                qsl = slice(qb * 512, (qb + 1) * 512)
                pA, pB = [], []
                for p in range(NPAIR):
                    scA = psA.tile([128, 2, 512], F32, tag="psA", name="scA")
                    scB = psA.tile([128, 2, 512], F32, tag="psA", name="scB")
                    for j in range(2):
                        kt = 2 * p + j
                        ksl = slice(kt * 128, (kt + 1) * 128)
                        nc.tensor.matmul(
                            scA[:, j, :], ck[0:64, ft, ksl], cq[0:64, ft, qsl],
                            start=True, stop=True,
                        )
                        nc.tensor.matmul(
                            scB[:, j, :], ck[64:128, ft, ksl], cq[64:128, ft, qsl],
                            start=True, stop=True,
                        )
                    prA = probs_pool.tile([128, 2, 512], BF16, tag="probs", name="prA")
                    nc.scalar.activation(
                        out=prA[:], in_=scA[:],
                        func=mybir.ActivationFunctionType.Exp,
                    )
                    prB = probs_pool.tile([128, 2, 512], BF16, tag="probs", name="prB")
                    nc.scalar.activation(
                        out=prB[:], in_=scB[:],
                        func=mybir.ActivationFunctionType.Exp,
                    )
                    pA.append(prA)
                    pB.append(prB)
                    pull(8)
                pairs_of[(hp, qb)] = (pA, pB)

            push_av(6)
            push_av(7)
            pull(10 ** 9)

    nc.finalize()
    return nc


def _get_nc():
    if "nc" not in _CACHE:
        _CACHE["nc"] = _build()
    return _CACHE["nc"]


def kernel(x, tokens, Wq, bq, Wk, bk, Wv, bv):
    x = np.asarray(x, dtype=np.float32)
    tokens = np.asarray(tokens, dtype=np.float32)
    Wq = np.asarray(Wq, dtype=np.float32)
    Wk = np.asarray(Wk, dtype=np.float32)
    Wv = np.asarray(Wv, dtype=np.float32)
    bq = np.asarray(bq, dtype=np.float32)
    bk = np.asarray(bk, dtype=np.float32)
    bv = np.asarray(bv, dtype=np.float32)

    bf16 = ml_dtypes.bfloat16
    in_maps = []
    for c in range(NCORES):
        b, g = divmod(c, 2)
        rows = slice(g * FPG, (g + 1) * FPG)
        tq = tokens[b, 0] @ Wq[rows].T + 2.0 * bq[rows]   # [512]
        tk = tokens[b, 0] @ Wk[rows].T + 2.0 * bk[rows]

        def packx(aT):
            # [D, C] -> [128, NKC, C]: partition-major to match SBUF layout
            return np.ascontiguousarray(
                aT.reshape(NKC, 128, aT.shape[1]).transpose(1, 0, 2)
            ).astype(bf16)

        def packw(aT):
            # [D, FPG] -> [128, NFT, NKC, 128]: ft-major
            return np.ascontiguousarray(
                aT.reshape(NKC, 128, NFT, 128).transpose(1, 2, 0, 3)
            ).astype(bf16)

        fq = packw(Wq[rows].T)
        fk = packw(Wk[rows].T)
        qa = (tq / 8.0).reshape(NFT, 128).T.astype(np.float32)   # [128, NFT]
        ka = tk.reshape(NFT, 128).T.astype(np.float32)

        in_maps.append({
            "xT": packx(x[b].T),
            "adds": np.ascontiguousarray(np.stack([qa, ka], axis=1)),
            "w0": np.ascontiguousarray(np.stack([fq[:, 0], fk[:, 0]], axis=1)),
            "wqk": np.ascontiguousarray(np.stack([fq[:, 1:], fk[:, 1:]], axis=1)),
            "wvT": packx(Wv[rows].T),
        })

    nc = _get_nc()
    trace = bool(int(os.environ.get("KERNEL_TRACE", "0")))
    res = run_bass_kernel_spmd(nc, in_maps, core_ids=list(range(NCORES)), trace=trace)
    if trace:
        _CACHE["last_results"] = res

    y = np.empty((B, S, D), dtype=np.float32)
    for c in range(NCORES):
        b, g = divmod(c, 2)
        blk = np.asarray(res.results[c]["yT"], dtype=np.float32)
        blk = blk.reshape(NFT, NQB, 128, 512)
        y[b, :, g * FPG:(g + 1) * FPG] = (
            blk.transpose(1, 3, 0, 2).reshape(S, FPG)
        )
    y += bv[None, None, :]
    return y
